# revision 19
# baseline (speedup 1.0000x reference)
"""Trainium2 Bass kernel for 16-head causal MultiHeadAttention.

Problem: B=2, S=2048, D=1024, H=16 heads of 64. Causal mask, softmax,
fp32 reference (computed in bf16 on the PE, fp32 PSUM accum; rel-err
budget 2e-2, measured ~6e-3).

Sharding: tensor-parallel over heads. Each of the 8 cores handles 2 heads
(a 128-wide feature slice): it computes Q/K/V projections for its slice,
causal attention for its 2 heads over both batch elements, and a partial
output projection y_c = A_c @ Wo[c*128:(c+1)*128, :]. The host sums the 8
partials and adds bo (the "unshard" step).

Schedule: the PE is strict-FIFO for matmuls, and attention alone leaves
the PE idle ~300ns per key tile (the exp on the scalar engine is the
per-tile critical chain at ~1us vs ~650ns of PE work). So all deferrable
PE work - the output projection of the previous chunk, the Q/K/V
projection matmul chains for a later chunk's tokens, and the V
transposes - is kept on a worklist and pumped between attention key
tiles, so those matmuls fill the exp-wait bubbles. All of it shares one
2-buffer PSUM pool; attention scores/accumulators use their own pools
(4 + 2 banks), totalling exactly the 8 PSUM banks.

Device layout notes (everything transposed, feature-on-partition):
  xT   [128, 8, 4096]   xT[p, kc, t] = x[t, kc*128+p]            (bf16)
  Q^T  [128, 4096]      rows = 2 heads x 64 feats, cols = token
  K^T  same. Scores are k=64 matmuls on the head's 64-partition slice;
                        the two heads auto-derive tile_position row
                        groups 0/64 and run concurrently in the PE array
  V    [128, 32, 2, 128] token-major slots per 128-token tile:
                        slot0 = [V_h0(64) | ones | 0...]  -> acc0 rows
                        0:64 = attn_h0, row 64 = rowsum_h0
                        slot1 = [ones | 0... | V_h1(64 @ cols 64:128)]
                        -> acc1 row 0 = rowsum_h1, rows 64:128 = attn_h1
                        (h1 lands directly on partitions 64:127, so no
                        partition-shift matmul is needed to assemble A^T)
  S^T  [keys, queries]  per (b, head, 512-query chunk), computed per
                        128-key tile; exp (scale=1/8, no max subtraction:
                        scores are ~N(0,1) so exp never overflows; masked
                        entries are multiplied by 0 afterwards, matching
                        the reference's -10000 masking whose exp
                        underflows to 0)
  normalize: both rowsums gathered into one SBUF tile, broadcast to all
  128 partitions with one k=128 selector matmul, then one full-width DVE
  reciprocal_approx_fast (~18 good bits; the custom op only honors base
  partition 0, hence recip-after-broadcast) and two DVE multiplies.
"""

import os
import sys
from collections import deque
from contextlib import ExitStack

import numpy as np

for _p in ("/opt/trn_rl_repo",):
    if _p not in sys.path and os.path.isdir(_p):
        sys.path.insert(0, _p)

import ml_dtypes

import concourse.bass as bass
import concourse.bacc as bacc
import concourse.tile as tile
from concourse import mybir
from concourse.bass import ts
from concourse.bass_utils import run_bass_kernel_spmd
from concourse.masks import make_identity

F32 = mybir.dt.float32
BF = mybir.dt.bfloat16
AF = mybir.ActivationFunctionType
BF_NP = ml_dtypes.bfloat16

B, S, D, H, HD = 2, 2048, 1024, 16, 64
T = B * S                     # 4096 tokens
NCORES = 8
FPC = D // NCORES             # 128 features per core (2 heads)
HPC = FPC // HD               # 2 heads per core
KC = D // 128                 # 8 contraction chunks for projections
TCH = T // 512                # 8 token chunks of 512
QCH = S // 512                # 4 query chunks per batch
NTT = T // 128                # 32 token tiles of 128


def build_nc() -> bass.Bass:
    nc = bacc.Bacc()

    xT = nc.declare_dram_parameter("xT", [128, KC, T], BF, False)
    wq = nc.declare_dram_parameter("wq", [128, KC, FPC], BF, False)
    wk = nc.declare_dram_parameter("wk", [128, KC, FPC], BF, False)
    wv = nc.declare_dram_parameter("wv", [128, KC, FPC], BF, False)
    wo = nc.declare_dram_parameter("wo", [FPC, D], BF, False)
    bq = nc.declare_dram_parameter("bq", [FPC, 1], F32, False)
    bk = nc.declare_dram_parameter("bk", [FPC, 1], F32, False)
    bv = nc.declare_dram_parameter("bv", [FPC, 1], F32, False)
    maskT = nc.declare_dram_parameter("maskT", [128, 4, 512], BF, False)
    yT = nc.declare_dram_parameter("yT", [D, T], BF, True)

    with tile.TileContext(nc) as tc, ExitStack() as ctx:
        const = ctx.enter_context(tc.tile_pool(name="const", bufs=1))
        constL = ctx.enter_context(tc.tile_pool(name="constL", bufs=1))
        persist = ctx.enter_context(tc.tile_pool(name="persist", bufs=1))
        xt_pool = ctx.enter_context(tc.tile_pool(name="xt_pool", bufs=2))
        vt_pool = ctx.enter_context(tc.tile_pool(name="vt_pool", bufs=2))
        pt_pool = ctx.enter_context(tc.tile_pool(name="pt_pool", bufs=4))
        yt_pool = ctx.enter_context(tc.tile_pool(name="yt_pool", bufs=3))
        rmat_pool = ctx.enter_context(tc.tile_pool(name="rmat_pool", bufs=2))

        wq_sb = const.tile([128, KC, FPC], BF)
        wk_sb = const.tile([128, KC, FPC], BF)
        wv_sb = const.tile([128, KC, FPC], BF)
        bq_sb = const.tile([FPC, 1], F32)
        bk_sb = const.tile([FPC, 1], F32)
        bv_sb = const.tile([FPC, 1], F32)
        ident = const.tile([128, 128], BF)
        sel2 = const.tile([128, 128], BF)
        wo_sb = constL.tile([FPC, D], BF)
        mask_sb = constL.tile([128, 4, 512], BF)
        # V-chain consts first: the first projection only needs wv/bv plus
        # the first xT chunk, so everything else loads in their shadow
        nc.scalar.dma_start(out=wv_sb, in_=wv[:])
        nc.scalar.dma_start(out=bv_sb, in_=bv[:])
        nc.scalar.dma_start(out=wk_sb, in_=wk[:])
        nc.scalar.dma_start(out=bk_sb, in_=bk[:])
        nc.scalar.dma_start(out=wq_sb, in_=wq[:])
        nc.scalar.dma_start(out=bq_sb, in_=bq[:])
        make_identity(nc, ident)
        # selector for the rowsum broadcast: rmat_ps = sel2^T @ rsrc puts
        # rsrc row 64 (rowsum_h0) on partitions 0:64 and rsrc row 0
        # (rowsum_h1) on partitions 64:128
        nc.vector.memset(sel2, 0.0)
        nc.vector.memset(sel2[HD : HD + 1, 0:HD], 1.0)
        nc.vector.memset(sel2[0:1, HD:128], 1.0)

        QT = persist.tile([128, T], BF)
        KT = persist.tile([128, T], BF)
        V = persist.tile([128, NTT, 2, 128], BF)
        AT = persist.tile([128, T], BF)
        rsrc = persist.tile([128, 512], BF)
        vslots = V  # [128, NTT, 2, 128]
        # V slot constants: zeros on the halves the V copies won't write,
        # ones columns that turn the PV matmul into fused attn+rowsum
        nc.gpsimd.memset(vslots[:, :, 0, HD:128], 0.0)
        nc.gpsimd.memset(vslots[:, :, 1, 0:HD], 0.0)
        nc.gpsimd.memset(vslots[:, :, 0, HD : HD + 1], 1.0)
        nc.gpsimd.memset(vslots[:, :, 1, 0:1], 1.0)
        # rsrc rows other than 0/64 are contracted by sel2 zeros but must
        # be finite
        nc.gpsimd.memset(rsrc, 0.0)
        nc.scalar.dma_start(out=wo_sb, in_=wo[:])
        nc.scalar.dma_start(out=mask_sb, in_=maskT[:])

        with (
            tc.tile_pool(name="st_ps", bufs=2, space="PSUM") as st_ps,
            tc.tile_pool(name="acc_ps", bufs=2, space="PSUM") as acc_ps,
            tc.tile_pool(name="work_ps", bufs=2, space="PSUM") as work_ps,
        ):
            # ---- worklist of deferred PE micro-ops (pumped between ----
            # ---- attention key tiles to fill exp-wait PE bubbles)  ----
            worklist = deque()  # (tag, fn) in FIFO order

            def pump(n):
                for _ in range(n):
                    if worklist:
                        worklist.popleft()[1]()

            def flush_proj():
                # run through the queue until no projection items remain;
                # output-projection items may ride into later chunks where
                # they fill exp-bound PE bubbles
                while any(t == "proj" for t, _ in worklist):
                    worklist.popleft()[1]()

            def push_outproj(g0):
                state = {}

                def op_mm(mt, g0=g0, state=state):
                    ps = work_ps.tile([128, 512], F32, name="work_ps")
                    nc.tensor.matmul(
                        ps,
                        wo_sb[:, ts(mt, 128)],
                        AT[:, g0 : g0 + 512],
                        start=True,
                        stop=True,
                    )
                    yt = yt_pool.tile([128, 512], BF, name="yt")
                    if mt == 6:
                        nc.scalar.activation(yt, ps, AF.Copy)
                    else:
                        nc.vector.tensor_copy(yt, ps)
                    nc.gpsimd.dma_start(
                        out=yT[ts(mt, 128), g0 : g0 + 512], in_=yt
                    )

                for mt in range(D // 128):
                    worklist.append(("op", lambda mt=mt: op_mm(mt)))

            def push_proj(tcn):
                # prefetch all of this chunk's x tiles now (sync queue)
                xts = []
                for kc in range(KC):
                    xt = xt_pool.tile([128, 512], BF, name=f"xt{kc}")
                    nc.sync.dma_start(out=xt, in_=xT[:, kc, ts(tcn, 512)])
                    xts.append(xt)

                def chain(wsb, drain, state):
                    def mm2(kc0, state=state):
                        if kc0 == 0:
                            state["ps"] = work_ps.tile(
                                [128, 512], F32, name="work_ps"
                            )
                        for kc in (kc0, kc0 + 1):
                            nc.tensor.matmul(
                                state["ps"],
                                wsb[:, kc, :],
                                xts[kc],
                                start=(kc == 0),
                                stop=(kc == KC - 1),
                            )

                    for kc0 in range(0, KC, 2):
                        worklist.append(("proj", lambda kc0=kc0: mm2(kc0)))
                    worklist.append(
                        ("proj", lambda state=state: drain(state["ps"]))
                    )

                # V chain first: its tail (transposes -> DVE copies) is the
                # deepest; K/Q drains go to the DVE to keep scalar on exp
                vstate = {}

                def vdrain(ps, tcn=tcn, vstate=vstate):
                    vt = vt_pool.tile([128, 512], BF)
                    nc.scalar.activation(vt, ps, AF.Identity, bias=bv_sb)
                    vstate["vt"] = vt

                chain(wv_sb, vdrain, vstate)

                def tr(i, tcn=tcn, vstate=vstate):
                    tp = work_ps.tile([128, 128], BF, name="work_ps")
                    nc.tensor.transpose(tp, vstate["vt"][:, ts(i, 128)], ident)
                    tt = tcn * 4 + i
                    # destination: slot0 cols 0:64 and slot1 cols 64:128
                    # (flat offsets 0:64 and 192:256 within this tile)
                    d0 = vslots[:, tt, 0, 0:HD]
                    dst = bass.AP(
                        tensor=d0.tensor,
                        offset=d0.offset,
                        ap=[list(d0.ap[0]), [192, 2], list(d0.ap[1])],
                    )
                    nc.vector.tensor_copy(
                        dst, tp.rearrange("p (g f) -> p g f", g=2)
                    )

                for i in range(4):
                    worklist.append(("proj", lambda i=i: tr(i)))

                chain(
                    wk_sb,
                    lambda ps, tcn=tcn: nc.vector.tensor_scalar_add(
                        KT[:, ts(tcn, 512)], ps, bk_sb
                    ),
                    {},
                )
                chain(
                    wq_sb,
                    lambda ps, tcn=tcn: nc.vector.tensor_scalar_add(
                        QT[:, ts(tcn, 512)], ps, bq_sb
                    ),
                    {},
                )

            pending = [None]

            def emit_attn(b, qc):
                g0 = b * S + qc * 512
                nkt = 4 * (qc + 1)  # causal: number of 128-key tiles
                acc0 = acc_ps.tile([128, 512], F32, name="accp", tag="accp")
                acc1 = acc_ps.tile([128, 512], F32, name="accp", tag="accp")
                accs = (acc0, acc1)
                # process the masked diagonal tiles first: their longer
                # exp -> mask -> PV chains overlap the unmasked tiles that
                # follow instead of delaying the normalize chain at the tail
                kt_order = list(range(4 * qc, nkt)) + list(range(0, 4 * qc))

                def emit_scores(kt):
                    k0 = b * S + kt * 128
                    st = st_ps.tile([128, 1024], F32, name="st_psum")
                    # k=64 per head; base partitions 0/64 auto-derive PE
                    # row-group tile positions, so the two run concurrently
                    for hl in range(2):
                        hs = slice(hl * HD, (hl + 1) * HD)
                        nc.tensor.matmul(
                            st[:, ts(hl, 512)],
                            KT[hs, k0 : k0 + 128],
                            QT[hs, g0 : g0 + 512],
                            start=True,
                            stop=True,
                        )
                    pt = pt_pool.tile([128, 1024], BF)
                    nc.scalar.activation(pt, st, AF.Exp, scale=0.125)
                    d = kt - 4 * qc
                    if d >= 0:
                        m = mask_sb[:, d, :]
                        m2 = bass.AP(
                            tensor=m.tensor,
                            offset=m.offset,
                            ap=[list(m.ap[0]), [0, 2], list(m.ap[1])],
                        )
                        nc.gpsimd.tensor_mul(
                            pt.rearrange("p (h j) -> p h j", h=2),
                            pt.rearrange("p (h j) -> p h j", h=2),
                            m2,
                        )
                    return pt

                def emit_pv(kt, pt, ktpos):
                    for hl in range(HPC):
                        nc.tensor.matmul(
                            accs[hl],
                            vslots[:, b * (S // 128) + kt, hl, :],
                            pt[:, ts(hl, 512)],
                            start=(ktpos == 0),
                            stop=(ktpos == nkt - 1),
                        )

                # PV lags scores by one tile so the next tile's scores and
                # pumped worklist matmuls sit between exp(k) and PV(k) in
                # the PE FIFO, covering the exp wait
                prev = None
                for ktpos, kt in enumerate(kt_order):
                    pt = emit_scores(kt)
                    if prev is not None:
                        pump(1)
                        emit_pv(prev[0], prev[1], ktpos - 1)
                    prev = (kt, pt)
                    pump(1)
                emit_pv(prev[0], prev[1], nkt - 1)
                flush_proj()  # next chunk needs its projections complete
                # normalize: gather both rowsums into SBUF, broadcast them
                # to all 128 partitions with one selector matmul, then one
                # full-width reciprocal (the custom DVE op only honors
                # base partition 0, so recip runs on the broadcast)
                nc.vector.tensor_copy(rsrc[HD : HD + 1, :], acc0[HD : HD + 1, :])
                nc.vector.tensor_copy(rsrc[0:1, :], acc1[0:1, :])
                rmat_ps = work_ps.tile([128, 512], F32, name="work_ps")
                nc.tensor.matmul(rmat_ps, sel2, rsrc, start=True, stop=True)
                rmat = rmat_pool.tile([128, 512], F32)
                nc.vector.reciprocal_approx_fast(out=rmat, in_=rmat_ps)
                nc.vector.tensor_mul(
                    AT[0:HD, g0 : g0 + 512], acc0[0:HD, :], rmat[0:HD, :]
                )
                nc.vector.tensor_mul(
                    AT[HD:128, g0 : g0 + 512], acc1[HD:128, :], rmat[HD:128, :]
                )

            # P0 runs directly; each attention chunk then pumps the next
            # projection chunk plus the previous chunk's output projection
            # through the worklist.
            push_proj(0)
            pump(len(worklist))
            for step in range(B * QCH):
                b, qc = divmod(step, QCH)
                if step + 1 < TCH:
                    push_proj(step + 1)
                if pending[0] is not None:
                    push_outproj(pending[0])
                emit_attn(b, qc)
                pending[0] = b * S + qc * 512
            push_outproj(pending[0])
            pump(len(worklist))

    nc.finalize()
    return nc


def _install_ntff_hook():
    """bass_utils' trace path needs antenv.axon_hooks, which this image's
    antenv lacks; synthesize it from the boot helper so NTFF profiling works."""
    try:
        from antenv.axon_hooks import get_axon_ntff_profile_hook  # noqa: F401

        return
    except ImportError:
        pass
    try:
        import types

        import antenv
        from trn_agent_boot.trn_boot import _ntff_profile_via_ctypes

        hook = _ntff_profile_via_ctypes("/opt/axon/libaxon_pjrt.so")
        mod = types.ModuleType("antenv.axon_hooks")
        state = {"hook": hook}
        mod.get_axon_ntff_profile_hook = lambda: state["hook"]
        mod.set_axon_ntff_profile_hook = lambda h: state.update(hook=h)
        sys.modules["antenv.axon_hooks"] = mod
        antenv.axon_hooks = mod
    except Exception:
        pass


_NC_CACHE: dict[str, bass.Bass] = {}


def _get_nc() -> bass.Bass:
    if "nc" not in _NC_CACHE:
        _NC_CACHE["nc"] = build_nc()
    return _NC_CACHE["nc"]


def _shard_inputs(inputs, Wq, bq, Wk, bk, Wv, bv, Wo, bo):
    x = np.ascontiguousarray(np.asarray(inputs, dtype=np.float32)).reshape(T, D)
    # xT[p, kc, t] = x[t, kc*128+p]
    xTh = np.ascontiguousarray(
        x.reshape(T, KC, 128).transpose(2, 1, 0).astype(BF_NP)
    )

    maskh = np.zeros((128, 4, 512), dtype=BF_NP)
    p = np.arange(128)[:, None]
    jj = np.arange(512)[None, :]
    for d in range(4):
        maskh[:, d, :] = (d * 128 + p <= jj).astype(BF_NP)

    def wslice(W, c):
        Wc = np.asarray(W, dtype=np.float32)[:, c * FPC : (c + 1) * FPC]
        # [128, KC, FPC] with [p, kc, m] = W[kc*128+p, m]
        return np.ascontiguousarray(
            Wc.reshape(KC, 128, FPC).transpose(1, 0, 2).astype(BF_NP)
        )

    in_maps = []
    for c in range(NCORES):
        in_maps.append(
            {
                "xT": xTh,
                "wq": wslice(Wq, c),
                "wk": wslice(Wk, c),
                "wv": wslice(Wv, c),
                "wo": np.ascontiguousarray(
                    np.asarray(Wo, dtype=np.float32)[
                        c * FPC : (c + 1) * FPC, :
                    ].astype(BF_NP)
                ),
                "bq": np.asarray(bq, np.float32)[c * FPC : (c + 1) * FPC, None],
                "bk": np.asarray(bk, np.float32)[c * FPC : (c + 1) * FPC, None],
                "bv": np.asarray(bv, np.float32)[c * FPC : (c + 1) * FPC, None],
                "maskT": maskh,
            }
        )
    return in_maps


def run_with_results(
    inputs,
    Wq,
    bq,
    Wk,
    bk,
    Wv,
    bv,
    Wo,
    bo,
    trace: bool = False,
):
    in_maps = _shard_inputs(inputs, Wq, bq, Wk, bk, Wv, bv, Wo, bo)
    if trace:
        _install_ntff_hook()
    nc = _get_nc()
    res = run_bass_kernel_spmd(
        nc, in_maps, core_ids=list(range(NCORES)), trace=trace
    )
    acc = np.zeros((D, T), dtype=np.float32)
    for c in range(NCORES):
        acc += res.results[c]["yT"].astype(np.float32)
    y = acc.T + np.asarray(bo, np.float32)[None, :]
    out = np.ascontiguousarray(y.reshape(B, S, D).astype(np.float32))
    return out, res


def kernel(**inputs) -> np.ndarray:
    out, _ = run_with_results(**inputs)
    return out


if __name__ == "__main__":
    nc = build_nc()
    print("built ok")


# revision 20
# speedup vs baseline: 1.2246x; 1.2246x over previous
"""Trainium2 Bass kernel for 16-head causal MultiHeadAttention.

Problem: B=2, S=2048, D=1024, H=16 heads of 64. Causal mask, softmax,
fp32 reference (computed in bf16 on the PE, fp32 PSUM accum; rel-err
budget 2e-2, measured ~6e-3).

Sharding: tensor-parallel over heads. Each of the 8 cores handles 2 heads
(a 128-wide feature slice): it computes Q/K/V projections for its slice,
causal attention for its 2 heads over both batch elements, and a partial
output projection y_c = A_c @ Wo[c*128:(c+1)*128, :]. The host sums the 8
partials and adds bo (the "unshard" step).

Schedule: the PE is strict-FIFO for matmuls, and attention alone leaves
the PE idle ~300ns per key tile (the exp on the scalar engine is the
per-tile critical chain at ~1us vs ~650ns of PE work). So all deferrable
PE work - the output projection of the previous chunk, the Q/K/V
projection matmul chains for a later chunk's tokens, and the V
transposes - is kept on a worklist and pumped between attention key
tiles, so those matmuls fill the exp-wait bubbles. All of it shares one
2-buffer PSUM pool; attention scores/accumulators use their own pools
(4 + 2 banks), totalling exactly the 8 PSUM banks.

Device layout notes (everything transposed, feature-on-partition):
  xT   [128, 8, 4096]   xT[p, kc, t] = x[t, kc*128+p]            (bf16)
  Q^T  [128, 4096]      rows = 2 heads x 64 feats, cols = token
  K^T  same. Scores are k=64 matmuls on the head's 64-partition slice;
                        the two heads auto-derive tile_position row
                        groups 0/64 and run concurrently in the PE array
  V    [128, 32, 2, 128] token-major slots per 128-token tile:
                        slot0 = [V_h0(64) | ones | 0...]  -> acc0 rows
                        0:64 = attn_h0, row 64 = rowsum_h0
                        slot1 = [ones | 0... | V_h1(64 @ cols 64:128)]
                        -> acc1 row 0 = rowsum_h1, rows 64:128 = attn_h1
                        (h1 lands directly on partitions 64:127, so no
                        partition-shift matmul is needed to assemble A^T)
  S^T  [keys, queries]  per (b, head, 512-query chunk), computed per
                        128-key tile; exp (scale=1/8, no max subtraction:
                        scores are ~N(0,1) so exp never overflows; masked
                        entries are multiplied by 0 afterwards, matching
                        the reference's -10000 masking whose exp
                        underflows to 0)
  normalize: both rowsums gathered into one SBUF tile, broadcast to all
  128 partitions with one k=128 selector matmul, then one full-width DVE
  reciprocal_approx_fast (~18 good bits; the custom op only honors base
  partition 0, hence recip-after-broadcast) and two DVE multiplies.
"""

import os
import sys
from collections import deque
from contextlib import ExitStack

import numpy as np

for _p in ("/opt/trn_rl_repo",):
    if _p not in sys.path and os.path.isdir(_p):
        sys.path.insert(0, _p)

import ml_dtypes

import concourse.bass as bass
import concourse.bacc as bacc
import concourse.tile as tile
from concourse import mybir
from concourse.bass import ts
from concourse.bass_utils import run_bass_kernel_spmd
from concourse.masks import make_identity

F32 = mybir.dt.float32
BF = mybir.dt.bfloat16
AF = mybir.ActivationFunctionType
BF_NP = ml_dtypes.bfloat16

B, S, D, H, HD = 2, 2048, 1024, 16, 64
T = B * S                     # 4096 tokens
NCORES = 8
FPC = D // NCORES             # 128 features per core (2 heads)
HPC = FPC // HD               # 2 heads per core
KC = D // 128                 # 8 contraction chunks for projections
TCH = T // 512                # 8 token chunks of 512
QCH = S // 512                # 4 query chunks per batch
NTT = T // 128                # 32 token tiles of 128


def build_nc() -> bass.Bass:
    nc = bacc.Bacc()

    xT = nc.declare_dram_parameter("xT", [128, KC, T], BF, False)
    wq = nc.declare_dram_parameter("wq", [128, KC, FPC], BF, False)
    wk = nc.declare_dram_parameter("wk", [128, KC, FPC], BF, False)
    wv = nc.declare_dram_parameter("wv", [128, KC, FPC], BF, False)
    wo = nc.declare_dram_parameter("wo", [FPC, D], BF, False)
    bq = nc.declare_dram_parameter("bq", [FPC, 1], F32, False)
    bk = nc.declare_dram_parameter("bk", [FPC, 1], F32, False)
    bv = nc.declare_dram_parameter("bv", [FPC, 1], F32, False)
    maskT = nc.declare_dram_parameter("maskT", [128, 4, 512], BF, False)
    yT = nc.declare_dram_parameter("yT", [D, T], BF, True)

    with tile.TileContext(nc) as tc, ExitStack() as ctx:
        const = ctx.enter_context(tc.tile_pool(name="const", bufs=1))
        constL = ctx.enter_context(tc.tile_pool(name="constL", bufs=1))
        persist = ctx.enter_context(tc.tile_pool(name="persist", bufs=1))
        xt_pool = ctx.enter_context(tc.tile_pool(name="xt_pool", bufs=2))
        vt_pool = ctx.enter_context(tc.tile_pool(name="vt_pool", bufs=2))
        pt_pool = ctx.enter_context(tc.tile_pool(name="pt_pool", bufs=4))
        yt_pool = ctx.enter_context(tc.tile_pool(name="yt_pool", bufs=3))
        rmat_pool = ctx.enter_context(tc.tile_pool(name="rmat_pool", bufs=2))

        wq_sb = const.tile([128, KC, FPC], BF)
        wk_sb = const.tile([128, KC, FPC], BF)
        wv_sb = const.tile([128, KC, FPC], BF)
        bq_sb = const.tile([FPC, 1], F32)
        bk_sb = const.tile([FPC, 1], F32)
        bv_sb = const.tile([FPC, 1], F32)
        ident = const.tile([128, 128], BF)
        sel2 = const.tile([128, 128], BF)
        wo_sb = constL.tile([FPC, D], BF)
        mask_sb = constL.tile([128, 4, 512], BF)
        # V-chain consts first: the first projection only needs wv/bv plus
        # the first xT chunk, so everything else loads in their shadow
        nc.scalar.dma_start(out=wv_sb, in_=wv[:])
        nc.scalar.dma_start(out=bv_sb, in_=bv[:])
        nc.scalar.dma_start(out=wk_sb, in_=wk[:])
        nc.scalar.dma_start(out=bk_sb, in_=bk[:])
        nc.scalar.dma_start(out=wq_sb, in_=wq[:])
        nc.scalar.dma_start(out=bq_sb, in_=bq[:])
        make_identity(nc, ident)
        # selector for the rowsum broadcast: rmat_ps = sel2^T @ rsrc puts
        # rsrc row 64 (rowsum_h0) on partitions 0:64 and rsrc row 0
        # (rowsum_h1) on partitions 64:128
        nc.vector.memset(sel2, 0.0)
        nc.vector.memset(sel2[HD : HD + 1, 0:HD], 1.0)
        nc.vector.memset(sel2[0:1, HD:128], 1.0)

        QT = persist.tile([128, T], BF)
        KT = persist.tile([128, T], BF)
        V = persist.tile([128, NTT, 2, 128], BF)
        AT = persist.tile([128, T], BF)
        rsrc = persist.tile([128, 512], BF)
        vslots = V  # [128, NTT, 2, 128]
        # V slot constants: zeros on the halves the V copies won't write,
        # ones columns that turn the PV matmul into fused attn+rowsum
        nc.gpsimd.memset(vslots[:, :, 0, HD:128], 0.0)
        nc.gpsimd.memset(vslots[:, :, 1, 0:HD], 0.0)
        nc.gpsimd.memset(vslots[:, :, 0, HD : HD + 1], 1.0)
        nc.gpsimd.memset(vslots[:, :, 1, 0:1], 1.0)
        # rsrc rows other than 0/64 are contracted by sel2 zeros but must
        # be finite
        nc.gpsimd.memset(rsrc, 0.0)
        nc.scalar.dma_start(out=wo_sb, in_=wo[:])
        nc.scalar.dma_start(out=mask_sb, in_=maskT[:])

        with (
            tc.tile_pool(name="st_ps", bufs=2, space="PSUM") as st_ps,
            tc.tile_pool(name="acc_ps", bufs=2, space="PSUM") as acc_ps,
            tc.tile_pool(name="work_ps", bufs=2, space="PSUM") as work_ps,
        ):
            # ---- worklist of deferred PE micro-ops (pumped between ----
            # ---- attention key tiles to fill exp-wait PE bubbles)  ----
            worklist = deque()  # (tag, fn) in FIFO order

            def pump(n):
                for _ in range(n):
                    if worklist:
                        worklist.popleft()[1]()

            def flush_proj():
                # run through the queue until no projection items remain;
                # output-projection items may ride into later chunks where
                # they fill exp-bound PE bubbles
                while any(t == "proj" for t, _ in worklist):
                    worklist.popleft()[1]()

            def push_outproj(g0):
                state = {}

                def op_mm(mt, g0=g0, state=state):
                    ps = work_ps.tile([128, 512], F32, name="work_ps")
                    nc.tensor.matmul(
                        ps,
                        wo_sb[:, ts(mt, 128)],
                        AT[:, g0 : g0 + 512],
                        start=True,
                        stop=True,
                    )
                    yt = yt_pool.tile([128, 512], BF, name="yt")
                    if mt == 6:
                        nc.scalar.activation(yt, ps, AF.Copy)
                    else:
                        nc.vector.tensor_copy(yt, ps)
                    nc.gpsimd.dma_start(
                        out=yT[ts(mt, 128), g0 : g0 + 512], in_=yt
                    )

                for mt in range(D // 128):
                    worklist.append(("op", lambda mt=mt: op_mm(mt)))

            def push_proj(tcn):
                # prefetch all of this chunk's x tiles now (sync queue)
                xts = []
                for kc in range(KC):
                    xt = xt_pool.tile([128, 512], BF, name=f"xt{kc}")
                    nc.sync.dma_start(out=xt, in_=xT[:, kc, ts(tcn, 512)])
                    xts.append(xt)

                def chain(wsb, drain, state):
                    def mm2(kc0, state=state):
                        if kc0 == 0:
                            state["ps"] = work_ps.tile(
                                [128, 512], F32, name="work_ps"
                            )
                        for kc in (kc0, kc0 + 1):
                            nc.tensor.matmul(
                                state["ps"],
                                wsb[:, kc, :],
                                xts[kc],
                                start=(kc == 0),
                                stop=(kc == KC - 1),
                            )

                    for kc0 in range(0, KC, 2):
                        worklist.append(("proj", lambda kc0=kc0: mm2(kc0)))
                    worklist.append(
                        ("proj", lambda state=state: drain(state["ps"]))
                    )

                # V chain first: its tail (transposes -> DVE copies) is the
                # deepest; K/Q drains go to the DVE to keep scalar on exp
                vstate = {}

                def vdrain(ps, tcn=tcn, vstate=vstate):
                    vt = vt_pool.tile([128, 512], BF)
                    nc.scalar.activation(vt, ps, AF.Identity, bias=bv_sb)
                    vstate["vt"] = vt

                chain(wv_sb, vdrain, vstate)

                def tr(i, tcn=tcn, vstate=vstate):
                    tp = work_ps.tile([128, 128], BF, name="work_ps")
                    nc.tensor.transpose(tp, vstate["vt"][:, ts(i, 128)], ident)
                    tt = tcn * 4 + i
                    # destination: slot0 cols 0:64 and slot1 cols 64:128
                    # (flat offsets 0:64 and 192:256 within this tile)
                    d0 = vslots[:, tt, 0, 0:HD]
                    dst = bass.AP(
                        tensor=d0.tensor,
                        offset=d0.offset,
                        ap=[list(d0.ap[0]), [192, 2], list(d0.ap[1])],
                    )
                    nc.vector.tensor_copy(
                        dst, tp.rearrange("p (g f) -> p g f", g=2)
                    )

                for i in range(4):
                    worklist.append(("proj", lambda i=i: tr(i)))

                chain(
                    wk_sb,
                    lambda ps, tcn=tcn: nc.vector.tensor_scalar_add(
                        KT[:, ts(tcn, 512)], ps, bk_sb
                    ),
                    {},
                )
                chain(
                    wq_sb,
                    lambda ps, tcn=tcn: nc.vector.tensor_scalar_add(
                        QT[:, ts(tcn, 512)], ps, bq_sb
                    ),
                    {},
                )

            pending = [None]

            def emit_attn(b, qc):
                g0 = b * S + qc * 512
                nkt = 4 * (qc + 1)  # causal: number of 128-key tiles
                acc0 = acc_ps.tile([128, 512], F32, name="accp", tag="accp")
                acc1 = acc_ps.tile([128, 512], F32, name="accp", tag="accp")
                accs = (acc0, acc1)
                # process the masked diagonal tiles first: their longer
                # exp -> mask -> PV chains overlap the unmasked tiles that
                # follow instead of delaying the normalize chain at the tail
                kt_order = list(range(4 * qc, nkt)) + list(range(0, 4 * qc))

                def emit_scores(kt):
                    k0 = b * S + kt * 128
                    st = st_ps.tile([128, 1024], F32, name="st_psum")
                    # k=64 per head; base partitions 0/64 auto-derive PE
                    # row-group tile positions, so the two run concurrently
                    for hl in range(2):
                        hs = slice(hl * HD, (hl + 1) * HD)
                        nc.tensor.matmul(
                            st[:, ts(hl, 512)],
                            KT[hs, k0 : k0 + 128],
                            QT[hs, g0 : g0 + 512],
                            start=True,
                            stop=True,
                        )
                    pt = pt_pool.tile([128, 1024], BF)
                    nc.scalar.activation(pt, st, AF.Exp, scale=0.125)
                    d = kt - 4 * qc
                    if d >= 0:
                        m = mask_sb[:, d, :]
                        m2 = bass.AP(
                            tensor=m.tensor,
                            offset=m.offset,
                            ap=[list(m.ap[0]), [0, 2], list(m.ap[1])],
                        )
                        nc.vector.tensor_mul(
                            pt.rearrange("p (h j) -> p h j", h=2),
                            pt.rearrange("p (h j) -> p h j", h=2),
                            m2,
                        )
                    return pt

                def emit_pv(kt, pt, ktpos):
                    for hl in range(HPC):
                        nc.tensor.matmul(
                            accs[hl],
                            vslots[:, b * (S // 128) + kt, hl, :],
                            pt[:, ts(hl, 512)],
                            start=(ktpos == 0),
                            stop=(ktpos == nkt - 1),
                        )

                # PV lags scores by one tile so the next tile's scores and
                # pumped worklist matmuls sit between exp(k) and PV(k) in
                # the PE FIFO, covering the exp wait
                prev = None
                for ktpos, kt in enumerate(kt_order):
                    pt = emit_scores(kt)
                    if prev is not None:
                        pump(1)
                        emit_pv(prev[0], prev[1], ktpos - 1)
                    prev = (kt, pt)
                    pump(1)
                emit_pv(prev[0], prev[1], nkt - 1)
                flush_proj()  # next chunk needs its projections complete
                # normalize: gather both rowsums into SBUF, broadcast them
                # to all 128 partitions with one selector matmul, then one
                # full-width reciprocal (the custom DVE op only honors
                # base partition 0, so recip runs on the broadcast)
                nc.vector.tensor_copy(rsrc[HD : HD + 1, :], acc0[HD : HD + 1, :])
                nc.vector.tensor_copy(rsrc[0:1, :], acc1[0:1, :])
                rmat_ps = work_ps.tile([128, 512], F32, name="work_ps")
                nc.tensor.matmul(rmat_ps, sel2, rsrc, start=True, stop=True)
                rmat = rmat_pool.tile([128, 512], F32)
                nc.vector.reciprocal_approx_fast(out=rmat, in_=rmat_ps)
                nc.vector.tensor_mul(
                    AT[0:HD, g0 : g0 + 512], acc0[0:HD, :], rmat[0:HD, :]
                )
                nc.vector.tensor_mul(
                    AT[HD:128, g0 : g0 + 512], acc1[HD:128, :], rmat[HD:128, :]
                )

            # P0 runs directly; each attention chunk then pumps the next
            # projection chunk plus the previous chunk's output projection
            # through the worklist.
            push_proj(0)
            pump(len(worklist))
            for step in range(B * QCH):
                b, qc = divmod(step, QCH)
                if step + 1 < TCH:
                    push_proj(step + 1)
                if pending[0] is not None:
                    push_outproj(pending[0])
                emit_attn(b, qc)
                pending[0] = b * S + qc * 512
            push_outproj(pending[0])
            pump(len(worklist))

    nc.finalize()
    return nc


def _install_ntff_hook():
    """bass_utils' trace path needs antenv.axon_hooks, which this image's
    antenv lacks; synthesize it from the boot helper so NTFF profiling works."""
    try:
        from antenv.axon_hooks import get_axon_ntff_profile_hook  # noqa: F401

        return
    except ImportError:
        pass
    try:
        import types

        import antenv
        from trn_agent_boot.trn_boot import _ntff_profile_via_ctypes

        hook = _ntff_profile_via_ctypes("/opt/axon/libaxon_pjrt.so")
        mod = types.ModuleType("antenv.axon_hooks")
        state = {"hook": hook}
        mod.get_axon_ntff_profile_hook = lambda: state["hook"]
        mod.set_axon_ntff_profile_hook = lambda h: state.update(hook=h)
        sys.modules["antenv.axon_hooks"] = mod
        antenv.axon_hooks = mod
    except Exception:
        pass


_NC_CACHE: dict[str, bass.Bass] = {}


def _get_nc() -> bass.Bass:
    if "nc" not in _NC_CACHE:
        _NC_CACHE["nc"] = build_nc()
    return _NC_CACHE["nc"]


def _shard_inputs(inputs, Wq, bq, Wk, bk, Wv, bv, Wo, bo):
    x = np.ascontiguousarray(np.asarray(inputs, dtype=np.float32)).reshape(T, D)
    # xT[p, kc, t] = x[t, kc*128+p]
    xTh = np.ascontiguousarray(
        x.reshape(T, KC, 128).transpose(2, 1, 0).astype(BF_NP)
    )

    maskh = np.zeros((128, 4, 512), dtype=BF_NP)
    p = np.arange(128)[:, None]
    jj = np.arange(512)[None, :]
    for d in range(4):
        maskh[:, d, :] = (d * 128 + p <= jj).astype(BF_NP)

    def wslice(W, c):
        Wc = np.asarray(W, dtype=np.float32)[:, c * FPC : (c + 1) * FPC]
        # [128, KC, FPC] with [p, kc, m] = W[kc*128+p, m]
        return np.ascontiguousarray(
            Wc.reshape(KC, 128, FPC).transpose(1, 0, 2).astype(BF_NP)
        )

    in_maps = []
    for c in range(NCORES):
        in_maps.append(
            {
                "xT": xTh,
                "wq": wslice(Wq, c),
                "wk": wslice(Wk, c),
                "wv": wslice(Wv, c),
                "wo": np.ascontiguousarray(
                    np.asarray(Wo, dtype=np.float32)[
                        c * FPC : (c + 1) * FPC, :
                    ].astype(BF_NP)
                ),
                "bq": np.asarray(bq, np.float32)[c * FPC : (c + 1) * FPC, None],
                "bk": np.asarray(bk, np.float32)[c * FPC : (c + 1) * FPC, None],
                "bv": np.asarray(bv, np.float32)[c * FPC : (c + 1) * FPC, None],
                "maskT": maskh,
            }
        )
    return in_maps


def run_with_results(
    inputs,
    Wq,
    bq,
    Wk,
    bk,
    Wv,
    bv,
    Wo,
    bo,
    trace: bool = False,
):
    in_maps = _shard_inputs(inputs, Wq, bq, Wk, bk, Wv, bv, Wo, bo)
    if trace:
        _install_ntff_hook()
    nc = _get_nc()
    res = run_bass_kernel_spmd(
        nc, in_maps, core_ids=list(range(NCORES)), trace=trace
    )
    acc = np.zeros((D, T), dtype=np.float32)
    for c in range(NCORES):
        acc += res.results[c]["yT"].astype(np.float32)
    y = acc.T + np.asarray(bo, np.float32)[None, :]
    out = np.ascontiguousarray(y.reshape(B, S, D).astype(np.float32))
    return out, res


def kernel(**inputs) -> np.ndarray:
    out, _ = run_with_results(**inputs)
    return out


if __name__ == "__main__":
    nc = build_nc()
    print("built ok")
